# revision 17
# baseline (speedup 1.0000x reference)
"""Trainium2 Bass kernel for BasicMPNN (gnn_message_passing), 8 NeuronCores.

Strategy (per sharding hint): shard nodes + incident edges across the 8
cores by destination-node range; replicate the small MLP weights; AllGather
the per-core message tables; AllReduce only the final per-graph pooled sums.

Pipeline per core c (owns nodes [c*6250, (c+1)*6250)):
  A. msg_own = relu(x_own @ Wm + bm)   -> local DRAM  [6272, 128]
     (x tiles PE-transposed into a persistent SBUF xT slab, reused in C)
  AG. AllGather msg_own -> msg_full [50176, 128] (row r of node n:
     r = (n // 6250) * 6272 + n % 6250)
  B. For each 128-dst-slot window: dma_gather (4 SWDGE queues, int16 idx,
     two calls against the two table halves) pulls the source messages of
     the window's in-edges into SBUF [edge, feat]; a DVE-built one-hot
     S[edge, dstslot] and PE matmul accumulate aggT[feat, dstslot] in PSUM.
  C. hT = relu(Wu_x.T-part @ xT + Wu_a.T-part @ aggT + bu)  [hid, node]
  D. per-node logit l = hT.T @ Wo; pooled = onehot(batch).T @ [l, 1]
     accumulated over all node tiles in PSUM [64, 2].
  AR. AllReduce pooled -> z = l_sum / max(count,1) + bo -> sigmoid -> out.
"""

import numpy as np

import concourse.bass as bass
import concourse.mybir as mybir
import concourse.tile as tile
import concourse.bacc as bacc
from concourse.bass_utils import run_bass_kernel_spmd
from concourse.masks import make_identity

F32 = mybir.dt.float32
BF16 = mybir.dt.bfloat16
I16 = mybir.dt.int16

N_NODES = 50000
N_EDGES = 800000
D = 128            # IN_DIM == HID_DIM == 128
OUT_DIM = 1
G = 64             # NUM_GRAPHS
NCORES = 8
NPC = N_NODES // NCORES          # 6250 nodes per core
NTILES = (NPC + 127) // 128      # 49 node tiles per core
NPAD = NTILES * 128              # 6272 padded rows per core
TROWS = NCORES * NPAD            # 50176 padded table rows
HALF = TROWS // 2                # 25088 = 4 cores * 6272 (int16-addressable)
PAD_SLOT = 255.0                 # one-hot code that matches nothing


def _pack_idx16(idx, nblk):
    """Pack a section's row indices (len nblk*128, int) into the dma_gather
    int16 layout: idx i -> [i % 16, i // 16], replicated 8x down partitions."""
    arr = np.asarray(idx, dtype=np.int16).reshape(nblk * 8, 16).T  # [16, nblk*8]
    return np.tile(arr, (8, 1))                                    # [128, nblk*8]


def _host_prep(x, edge_index, batch):
    """Pure-numpy graph partitioning -> per-core input arrays + schedule."""
    src = np.asarray(edge_index[1], dtype=np.int64)
    dst = np.asarray(edge_index[0], dtype=np.int64)
    batch = np.asarray(batch, dtype=np.int64)
    rows = (src // NPC) * NPAD + (src % NPC)
    in_lo = rows < HALF

    core_of = dst // NPC
    src_core = src // NPC
    local_row = src % NPC            # row within the owner's msg_own [NPAD]
    # per (core, window, section): 0=own-core src, 1=remote lo, 2=remote hi
    per = [[[None] * 3 for _ in range(NTILES)] for _ in range(NCORES)]
    for c in range(NCORES):
        m = core_of == c
        ed, er = dst[m] - c * NPC, rows[m]
        own = src_core[m] == c
        lrow = local_row[m]
        win = ed >> 7
        slot = ed & 127
        lo = in_lo[m]
        for w in range(NTILES):
            wm = win == w
            per[c][w][0] = (lrow[wm & own], slot[wm & own])
            per[c][w][1] = (er[wm & ~own & lo], slot[wm & ~own & lo])
            per[c][w][2] = (er[wm & ~own & ~lo] - HALF, slot[wm & ~own & ~lo])

    # shared (compile-time) schedule: blocks per (window, section)
    B = np.zeros((NTILES, 3), np.int64)
    for w in range(NTILES):
        for s in range(3):
            n = max(max(len(per[c][w][s][0]) for c in range(NCORES)), 1)
            B[w, s] = (n + 127) // 128
    btot = B.sum(axis=1)                      # blocks per window
    IDXCOLS = int(B.sum()) * 8                # int16 idx cols total
    LDCOLS = int(btot.sum())                  # ld cols total

    gsrc = np.zeros((NCORES, 128, IDXCOLS), np.int16)
    ld = np.full((NCORES, 128, LDCOLS), PAD_SLOT, np.float32)
    io_off, ld_off = 0, 0
    sched = []                # (w, (B_own,B_lo,B_hi), (off_own,off_lo,off_hi), ld_off)
    for w in range(NTILES):
        offs = []
        for s in range(3):
            nb = int(B[w, s])
            for c in range(NCORES):
                er, slot = per[c][w][s]
                n = len(er)
                sec_idx = np.zeros(nb * 128, np.int64)
                sec_idx[:n] = er
                gsrc[c, :, io_off:io_off + nb * 8] = _pack_idx16(sec_idx, nb)
                # position i = b*128 + p  ->  ld[:, ld_off + prev + b]
                sl = np.full(nb * 128, PAD_SLOT, np.float32)
                sl[:n] = slot
                base = ld_off + int(B[w, :s].sum())
                ld[c, :, base:base + nb] = sl.reshape(nb, 128).T
            offs.append(io_off)
            io_off += nb * 8
        sched.append((w, (int(B[w, 0]), int(B[w, 1]), int(B[w, 2])),
                      tuple(offs), ld_off))
        ld_off += int(btot[w])

    sched.sort(key=lambda e: -sum(e[1]))      # big windows first
    # per-core padded transposed x and batch codes
    x = np.asarray(x, dtype=np.float32)
    x_own = np.zeros((NCORES, 128, NPAD), np.float32)
    batchf = np.full((NCORES, 128, NTILES), PAD_SLOT, np.float32)
    for c in range(NCORES):
        x_own[c, :, :NPC] = x[c * NPC:(c + 1) * NPC].T
        bt = np.full(NPAD, PAD_SLOT, np.float32)
        bt[:NPC] = batch[c * NPC:(c + 1) * NPC]
        batchf[c] = bt.reshape(NTILES, 128).T
    return gsrc, ld, batchf, x_own, sched, IDXCOLS, LDCOLS


def _build(sched, IDXCOLS, LDCOLS):
    nc = bacc.Bacc(None, target_bir_lowering=False, num_swdge_queues=4)
    xT_own = nc.dram_tensor("xT_own", [128, NPAD], F32, kind="ExternalInput")
    gsrc = nc.dram_tensor("gsrc", [128, IDXCOLS], I16, kind="ExternalInput")
    ldt_d = nc.dram_tensor("ld", [128, LDCOLS], F32, kind="ExternalInput")
    batchf = nc.dram_tensor("batchf", [128, NTILES], F32, kind="ExternalInput")
    Wm = nc.dram_tensor("Wm", [D, D], F32, kind="ExternalInput")
    bm_row = nc.dram_tensor("bm_row", [1, D], F32, kind="ExternalInput")
    bm_tile = nc.dram_tensor("bm_tile", [128, D], F32, kind="ExternalInput")
    Wu_x = nc.dram_tensor("Wu_x", [D, D], F32, kind="ExternalInput")
    Wu_a = nc.dram_tensor("Wu_a", [D, D], F32, kind="ExternalInput")
    bu_col = nc.dram_tensor("bu_col", [D, 1], F32, kind="ExternalInput")
    Wo_col = nc.dram_tensor("Wo_col", [D, 1], F32, kind="ExternalInput")
    bo_col = nc.dram_tensor("bo_col", [G, 1], F32, kind="ExternalInput")
    iota = nc.dram_tensor("iota", [128, 128], F32, kind="ExternalInput")
    ones_row = nc.dram_tensor("ones_row", [1, D], F32, kind="ExternalInput")
    ones_col = nc.dram_tensor("ones_col", [128, 1], F32, kind="ExternalInput")
    out = nc.dram_tensor("out", [G, OUT_DIM], F32, kind="ExternalOutput")

    with tile.TileContext(nc) as tc:
        with (
            tc.tile_pool(name="cst", bufs=1) as cst,
            tc.tile_pool(name="agp", bufs=NTILES) as agp,
            tc.tile_pool(name="slab", bufs=1) as slab,
            tc.tile_pool(name="gp", bufs=8) as gp,
            tc.tile_pool(name="gop", bufs=27) as gop,
            tc.tile_pool(name="sp", bufs=8) as sp,
            tc.tile_pool(name="cd", bufs=4) as cd,
            tc.tile_pool(name="ps", bufs=4, space="PSUM") as ps,
            tc.tile_pool(name="psB", bufs=3, space="PSUM") as psB,
            tc.tile_pool(name="pool_ps", bufs=1, space="PSUM") as pool_ps,
            tc.tile_pool(name="dram", bufs=1, space="DRAM") as dram,
        ):
            # constants (loaded once)
            iot = cst.tile([128, 128], F32)
            nc.sync.dma_start(out=iot[:], in_=iota[:])
            wm_t = cst.tile([D, D], F32)
            nc.sync.dma_start(out=wm_t[:], in_=Wm[:])
            bm_t = cst.tile([1, D], F32)
            nc.sync.dma_start(out=bm_t[:], in_=bm_row[:])
            bmt_t = cst.tile([128, D], F32)
            nc.sync.dma_start(out=bmt_t[:], in_=bm_tile[:])
            one_r = cst.tile([1, D], F32)
            nc.sync.dma_start(out=one_r[:], in_=ones_row[:])
            one_c = cst.tile([128, 1], F32)
            nc.sync.dma_start(out=one_c[:], in_=ones_col[:])
            wux_t = cst.tile([D, D], F32)
            nc.sync.dma_start(out=wux_t[:], in_=Wu_x[:])
            wua_t = cst.tile([D, D], F32)
            nc.sync.dma_start(out=wua_t[:], in_=Wu_a[:])
            bu_t = cst.tile([D, 1], F32)
            nc.sync.dma_start(out=bu_t[:], in_=bu_col[:])
            wo_t = cst.tile([D, 1], F32)
            nc.sync.dma_start(out=wo_t[:], in_=Wo_col[:])
            bo_t = cst.tile([G, 1], F32)
            nc.sync.dma_start(out=bo_t[:], in_=bo_col[:])
            # bulk-loaded per-core data
            gsrc_all = cst.tile([128, IDXCOLS], I16)
            nc.sync.dma_start(out=gsrc_all[:], in_=gsrc[:])
            ld_all = cst.tile([128, LDCOLS], F32)
            nc.sync.dma_start(out=ld_all[:], in_=ldt_d[:])
            bat_all = cst.tile([128, NTILES], F32)
            nc.sync.dma_start(out=bat_all[:], in_=batchf[:])

            xs = slab.tile([128, NPAD], F32)
            nc.sync.dma_start(out=xs[:], in_=xT_own[:])
            xT = [xs[:, t * 128:(t + 1) * 128] for t in range(NTILES)]
            mslab = slab.tile([128, NPAD], F32)
            aggT = [agp.tile([128, 128], F32, tag="aT", name=f"aT{t}")
                    for t in range(NTILES)]

            msg_own = dram.tile([NPAD, D], F32)
            msg_full = dram.tile([TROWS, D], F32, addr_space="Shared")
            ar_in = dram.tile([G, 2], F32)
            ar_out = dram.tile([G, 2], F32, addr_space="Shared")

            # ---- Phase A: msg_own = relu(x_own @ Wm + bm) ----
            for t in range(NTILES):
                pm = ps.tile([128, D], F32, tag="p128")
                nc.tensor.matmul(out=pm[:], lhsT=xT[t],
                                 rhs=wm_t[:], start=True, stop=True)
                mdst = mslab[:, t * 128:(t + 1) * 128]
                nc.vector.tensor_tensor(out=mdst, in0=pm[:], in1=bmt_t[:],
                                        op=mybir.AluOpType.add)
                nc.vector.tensor_scalar_max(out=mdst, in0=mdst, scalar1=0.0)
            # batched store in two chunks so the AllGather can start sooner
            HN = (NTILES // 2) * 128
            nc.sync.dma_start(
                out=msg_own[:HN].rearrange("(t p) d -> p t d", p=128),
                in_=mslab[:, :HN].rearrange("p (t d) -> p t d", d=D))
            nc.sync.dma_start(
                out=msg_own[HN:].rearrange("(t p) d -> p t d", p=128),
                in_=mslab[:, HN:].rearrange("p (t d) -> p t d", d=D))

            # ---- AllGather message table ----
            nc.gpsimd.collective_compute(
                "AllGather", mybir.AluOpType.bypass,
                replica_groups=[list(range(NCORES))],
                ins=[msg_own[:]], outs=[msg_full[:]])

            # ---- Phase B + interleaved C/D ----
            ppool = pool_ps.tile([G, 2], F32)
            qstate = [0]

            def gather(nb, off, sec, tag="g"):
                pool = gop if tag == "go" else gp
                g = pool.tile([128, nb * D], F32, tag=tag,
                              name=f"g{qstate[0]}")
                src_t = (msg_own[:], msg_full[:HALF], msg_full[HALF:])[sec]
                nc.gpsimd.dma_gather(
                    out_ap=g[:].rearrange("p (b d) -> p b d", d=D),
                    in_ap=src_t, idxs_ap=gsrc_all[:, off:off + nb * 8],
                    num_idxs=nb * 128, num_idxs_reg=nb * 128,
                    elem_size=D, single_packet=False,
                    queue_num=qstate[0] % 4)
                qstate[0] += 1
                return g

            # park the first PARK windows' own-core sections: these only
            # need msg_own, so they gen while the AllGather is in flight
            PARK = 24
            gown = {}
            for i in range(min(PARK, NTILES)):
                (w, (b_own, _, _), (off_own, _, _), _) = sched[i]
                gown[w] = gather(b_own, off_own, 0, tag="go")
            for i, (w, (b_own, b_lo, b_hi), (off_own, off_lo, off_hi),
                    ld_off) in enumerate(sched):
                if w not in gown:
                    gown[w] = gather(b_own, off_own, 0, tag="go")
                g_lo = gather(b_lo, off_lo, 1)
                g_hi = gather(b_hi, off_hi, 2)
                j = i + PARK
                if i >= min(PARK, NTILES) and j < NTILES:
                    (wn, (nb_own, _, _), (noff_own, _, _), _) = sched[j]
                    gown[wn] = gather(nb_own, noff_own, 0, tag="go")
                secs = [(gown.pop(w), b_own), (g_lo, b_lo), (g_hi, b_hi)]
                b_tot = b_own + b_lo + b_hi
                pagg = psB.tile([128, 128], F32, tag="pagg")
                b = 0
                for (g, nb) in secs:
                    for j in range(nb):
                        s = sp.tile([128, 128], F32, tag="s")
                        col = ld_off + b
                        nc.vector.tensor_tensor(
                            out=s[:],
                            in0=ld_all[:, col:col + 1].to_broadcast([128, 128]),
                            in1=iot[:], op=mybir.AluOpType.is_equal)
                        nc.tensor.matmul(out=pagg[:], lhsT=g[:, j * D:(j + 1) * D],
                                         rhs=s[:], start=(b == 0),
                                         stop=(b == b_tot - 1))
                        b += 1
                nc.vector.tensor_copy(out=aggT[w][:], in_=pagg[:])

                # ---- interleaved Phase C/D for node tile w ----
                t = w
                ph = ps.tile([128, 128], F32, tag="p128")
                nc.tensor.matmul(out=ph[:], lhsT=wux_t[:], rhs=xT[t],
                                 start=True, stop=False)
                nc.tensor.matmul(out=ph[:], lhsT=wua_t[:], rhs=aggT[t][:],
                                 start=False, stop=True)
                ht = cd.tile([128, 128], F32, tag="ht")
                nc.vector.tensor_scalar(out=ht[:], in0=ph[:],
                                        scalar1=bu_t[:, 0:1], scalar2=0.0,
                                        op0=mybir.AluOpType.add,
                                        op1=mybir.AluOpType.max)
                pl = ps.tile([128, 1], F32, tag="p128")
                nc.tensor.matmul(out=pl[:], lhsT=ht[:], rhs=wo_t[:],
                                 start=True, stop=True)
                r2 = cd.tile([128, 2], F32, tag="r2")
                nc.vector.tensor_copy(out=r2[:, 0:1], in_=pl[:])
                nc.vector.tensor_copy(out=r2[:, 1:2], in_=one_c[:])
                sb = sp.tile([128, G], F32, tag="sb")
                nc.vector.tensor_tensor(
                    out=sb[:], in0=bat_all[:, t:t + 1].to_broadcast([128, G]),
                    in1=iot[:, :G], op=mybir.AluOpType.is_equal)
                nc.tensor.matmul(out=ppool[:], lhsT=sb[:], rhs=r2[:],
                                 start=(t == 0), stop=(t == NTILES - 1))

            # ---- AllReduce pooled [64, 2], finish on every core ----
            psb = cd.tile([G, 2], F32, tag="psb")
            nc.vector.tensor_copy(out=psb[:], in_=ppool[:])
            nc.sync.dma_start(out=ar_in[:], in_=psb[:])
            nc.gpsimd.collective_compute(
                "AllReduce", mybir.AluOpType.add,
                replica_groups=[list(range(NCORES))],
                ins=[ar_in[:]], outs=[ar_out[:]])
            fin = cd.tile([G, 2], F32, tag="fin")
            nc.sync.dma_start(out=fin[:], in_=ar_out[:])
            cnt = cd.tile([G, 1], F32, tag="cnt")
            nc.vector.tensor_scalar_max(out=cnt[:], in0=fin[:, 1:2], scalar1=1.0)
            rcp = cd.tile([G, 1], F32, tag="rcp")
            nc.vector.reciprocal(out=rcp[:], in_=cnt[:])
            z = cd.tile([G, 1], F32, tag="z")
            nc.vector.tensor_tensor(out=z[:], in0=fin[:, 0:1], in1=rcp[:],
                                    op=mybir.AluOpType.mult)
            ot = cd.tile([G, 1], F32, tag="ot")
            nc.scalar.activation(out=ot[:], in_=z[:],
                                 func=mybir.ActivationFunctionType.Sigmoid,
                                 bias=bo_t[:])
            nc.sync.dma_start(out=out[:], in_=ot[:])
    nc.finalize()
    return nc


def _run(inputs, trace=False):
    x = np.asarray(inputs["x"], np.float32)
    Wm = np.asarray(inputs["Wm"], np.float32)
    bm = np.asarray(inputs["bm"], np.float32)
    Wu = np.asarray(inputs["Wu"], np.float32)
    bu = np.asarray(inputs["bu"], np.float32)
    Wo = np.asarray(inputs["Wo"], np.float32)
    bo = np.asarray(inputs["bo"], np.float32)

    gsrc, ld, batchf, x_own, sched, IDXCOLS, LDCOLS = _host_prep(
        x, inputs["edge_index"], inputs["batch"])
    nc = _build(sched, IDXCOLS, LDCOLS)

    iota = np.tile(np.arange(128, dtype=np.float32), (128, 1))
    common = {
        "Wm": Wm, "bm_row": bm.reshape(1, D),
        "bm_tile": np.tile(bm.reshape(1, D), (128, 1)),
        "Wu_x": Wu[:D], "Wu_a": Wu[D:], "bu_col": bu.reshape(D, 1),
        "Wo_col": Wo.reshape(D, 1),
        "bo_col": np.full((G, 1), float(bo.reshape(-1)[0]), np.float32),
        "iota": iota,
        "ones_row": np.ones((1, D), np.float32),
        "ones_col": np.ones((128, 1), np.float32),
    }
    in_maps = [dict(common, xT_own=x_own[c], gsrc=gsrc[c], ld=ld[c],
                    batchf=batchf[c]) for c in range(NCORES)]
    res = run_bass_kernel_spmd(nc, in_maps, list(range(NCORES)), trace=trace)
    return res.results[0]["out"].reshape(G, OUT_DIM), res


def kernel(**inputs) -> np.ndarray:
    out, _ = _run(inputs, trace=False)
    return out


# revision 18
# speedup vs baseline: 1.5080x; 1.5080x over previous
"""Trainium2 Bass kernel for BasicMPNN (gnn_message_passing), 8 NeuronCores.

Strategy (per sharding hint): shard nodes + incident edges across the 8
cores by destination-node range; replicate the small MLP weights; AllGather
the per-core message tables; AllReduce only the final per-graph pooled sums.

Pipeline per core c (owns nodes [c*6250, (c+1)*6250)):
  A. msg_own = relu(x_own @ Wm + bm)   -> local DRAM  [6272, 128]
     (x tiles PE-transposed into a persistent SBUF xT slab, reused in C)
  AG. AllGather msg_own -> msg_full [50176, 128] (row r of node n:
     r = (n // 6250) * 6272 + n % 6250)
  B. For each 128-dst-slot window: dma_gather (4 SWDGE queues, int16 idx,
     two calls against the two table halves) pulls the source messages of
     the window's in-edges into SBUF [edge, feat]; a DVE-built one-hot
     S[edge, dstslot] and PE matmul accumulate aggT[feat, dstslot] in PSUM.
  C. hT = relu(Wu_x.T-part @ xT + Wu_a.T-part @ aggT + bu)  [hid, node]
  D. per-node logit l = hT.T @ Wo; pooled = onehot(batch).T @ [l, 1]
     accumulated over all node tiles in PSUM [64, 2].
  AR. AllReduce pooled -> z = l_sum / max(count,1) + bo -> sigmoid -> out.
"""

import numpy as np

import concourse.bass as bass
import concourse.mybir as mybir
import concourse.tile as tile
import concourse.bacc as bacc
from concourse.bass_utils import run_bass_kernel_spmd
from concourse.masks import make_identity

F32 = mybir.dt.float32
BF16 = mybir.dt.bfloat16
I16 = mybir.dt.int16

N_NODES = 50000
N_EDGES = 800000
D = 128            # IN_DIM == HID_DIM == 128
OUT_DIM = 1
G = 64             # NUM_GRAPHS
NCORES = 8
NPC = N_NODES // NCORES          # 6250 nodes per core
NTILES = (NPC + 127) // 128      # 49 node tiles per core
NPAD = NTILES * 128              # 6272 padded rows per core
TROWS = NCORES * NPAD            # 50176 padded table rows
HALF = TROWS // 2                # 25088 = 4 cores * 6272 (int16-addressable)
PAD_SLOT = 255.0                 # one-hot code that matches nothing


def _pack_idx16(idx, nblk):
    """Pack a section's row indices (len nblk*128, int) into the dma_gather
    int16 layout: idx i -> [i % 16, i // 16], replicated 8x down partitions."""
    arr = np.asarray(idx, dtype=np.int16).reshape(nblk * 8, 16).T  # [16, nblk*8]
    return np.tile(arr, (8, 1))                                    # [128, nblk*8]


def _host_prep(x, edge_index, batch):
    """Pure-numpy graph partitioning -> per-core input arrays + schedule."""
    src = np.asarray(edge_index[1], dtype=np.int64)
    dst = np.asarray(edge_index[0], dtype=np.int64)
    batch = np.asarray(batch, dtype=np.int64)
    rows = (src // NPC) * NPAD + (src % NPC)
    in_lo = rows < HALF

    core_of = dst // NPC
    # per (core, window, section) edge lists; section 0: row < HALF
    per = [[[None, None] for _ in range(NTILES)] for _ in range(NCORES)]
    for c in range(NCORES):
        m = core_of == c
        ed, er = dst[m] - c * NPC, rows[m]
        win = ed >> 7
        slot = ed & 127
        lo = in_lo[m]
        for w in range(NTILES):
            wm = win == w
            for s, sm in ((0, wm & lo), (1, wm & ~lo)):
                per[c][w][s] = (er[sm] - (0 if s == 0 else HALF), slot[sm])

    # shared (compile-time) schedule: blocks per (window, section)
    B = np.zeros((NTILES, 2), np.int64)
    for w in range(NTILES):
        for s in range(2):
            n = max(max(len(per[c][w][s][0]) for c in range(NCORES)), 1)
            B[w, s] = (n + 127) // 128
    btot = B.sum(axis=1)                      # blocks per window
    IDXCOLS = int(B.sum()) * 8                # int16 idx cols total
    LDCOLS = int(btot.sum())                  # ld cols total

    gsrc = np.zeros((NCORES, 128, IDXCOLS), np.int16)
    ld = np.full((NCORES, 128, LDCOLS), PAD_SLOT, np.float32)
    io_off, ld_off = 0, 0
    sched = []                # (w, (B_lo, B_hi), (off_lo, off_hi), ld_off)
    for w in range(NTILES):
        offs = []
        for s in range(2):
            nb = int(B[w, s])
            for c in range(NCORES):
                er, slot = per[c][w][s]
                n = len(er)
                sec_idx = np.zeros(nb * 128, np.int64)
                sec_idx[:n] = er
                gsrc[c, :, io_off:io_off + nb * 8] = _pack_idx16(sec_idx, nb)
                # position i = b*128 + p  ->  ld[:, ld_off + prev + b]
                sl = np.full(nb * 128, PAD_SLOT, np.float32)
                sl[:n] = slot
                base = ld_off + int(B[w, :s].sum())
                ld[c, :, base:base + nb] = sl.reshape(nb, 128).T
            offs.append(io_off)
            io_off += nb * 8
        sched.append((w, (int(B[w, 0]), int(B[w, 1])), tuple(offs), ld_off))
        ld_off += int(btot[w])

    sched.sort(key=lambda e: -sum(e[1]))      # big windows first
    # per-core padded transposed x and batch codes
    x = np.asarray(x, dtype=np.float32)
    x_own = np.zeros((NCORES, 128, NPAD), np.float32)
    batchf = np.full((NCORES, 128, NTILES), PAD_SLOT, np.float32)
    for c in range(NCORES):
        x_own[c, :, :NPC] = x[c * NPC:(c + 1) * NPC].T
        bt = np.full(NPAD, PAD_SLOT, np.float32)
        bt[:NPC] = batch[c * NPC:(c + 1) * NPC]
        batchf[c] = bt.reshape(NTILES, 128).T
    return gsrc, ld, batchf, x_own, sched, IDXCOLS, LDCOLS


def _build(sched, IDXCOLS, LDCOLS):
    nc = bacc.Bacc(None, target_bir_lowering=False, num_swdge_queues=4)
    xT_own = nc.dram_tensor("xT_own", [128, NPAD], F32, kind="ExternalInput")
    gsrc = nc.dram_tensor("gsrc", [128, IDXCOLS], I16, kind="ExternalInput")
    ldt_d = nc.dram_tensor("ld", [128, LDCOLS], F32, kind="ExternalInput")
    batchf = nc.dram_tensor("batchf", [128, NTILES], F32, kind="ExternalInput")
    Wm = nc.dram_tensor("Wm", [D, D], F32, kind="ExternalInput")
    bm_row = nc.dram_tensor("bm_row", [1, D], F32, kind="ExternalInput")
    bm_tile = nc.dram_tensor("bm_tile", [128, D], F32, kind="ExternalInput")
    Wu_x = nc.dram_tensor("Wu_x", [D, D], F32, kind="ExternalInput")
    Wu_a = nc.dram_tensor("Wu_a", [D, D], F32, kind="ExternalInput")
    bu_col = nc.dram_tensor("bu_col", [D, 1], F32, kind="ExternalInput")
    Wo_col = nc.dram_tensor("Wo_col", [D, 1], F32, kind="ExternalInput")
    bo_col = nc.dram_tensor("bo_col", [G, 1], F32, kind="ExternalInput")
    iota = nc.dram_tensor("iota", [128, 128], F32, kind="ExternalInput")
    ones_row = nc.dram_tensor("ones_row", [1, D], F32, kind="ExternalInput")
    ones_col = nc.dram_tensor("ones_col", [128, 1], F32, kind="ExternalInput")
    out = nc.dram_tensor("out", [G, OUT_DIM], F32, kind="ExternalOutput")

    with tile.TileContext(nc) as tc:
        with (
            tc.tile_pool(name="cst", bufs=1) as cst,
            tc.tile_pool(name="agp", bufs=NTILES) as agp,
            tc.tile_pool(name="slab", bufs=1) as slab,
            tc.tile_pool(name="gp", bufs=8) as gp,
            tc.tile_pool(name="sp", bufs=8) as sp,
            tc.tile_pool(name="cd", bufs=4) as cd,
            tc.tile_pool(name="ps", bufs=4, space="PSUM") as ps,
            tc.tile_pool(name="psB", bufs=3, space="PSUM") as psB,
            tc.tile_pool(name="pool_ps", bufs=1, space="PSUM") as pool_ps,
            tc.tile_pool(name="dram", bufs=1, space="DRAM") as dram,
        ):
            # constants (loaded once)
            iot = cst.tile([128, 128], F32)
            nc.sync.dma_start(out=iot[:], in_=iota[:])
            wm_t = cst.tile([D, D], F32)
            nc.sync.dma_start(out=wm_t[:], in_=Wm[:])
            bm_t = cst.tile([1, D], F32)
            nc.sync.dma_start(out=bm_t[:], in_=bm_row[:])
            bmt_t = cst.tile([128, D], F32)
            nc.sync.dma_start(out=bmt_t[:], in_=bm_tile[:])
            one_r = cst.tile([1, D], F32)
            nc.sync.dma_start(out=one_r[:], in_=ones_row[:])
            one_c = cst.tile([128, 1], F32)
            nc.sync.dma_start(out=one_c[:], in_=ones_col[:])
            wux_t = cst.tile([D, D], F32)
            nc.sync.dma_start(out=wux_t[:], in_=Wu_x[:])
            wua_t = cst.tile([D, D], F32)
            nc.sync.dma_start(out=wua_t[:], in_=Wu_a[:])
            bu_t = cst.tile([D, 1], F32)
            nc.sync.dma_start(out=bu_t[:], in_=bu_col[:])
            wo_t = cst.tile([D, 1], F32)
            nc.sync.dma_start(out=wo_t[:], in_=Wo_col[:])
            bo_t = cst.tile([G, 1], F32)
            nc.sync.dma_start(out=bo_t[:], in_=bo_col[:])
            # bulk-loaded per-core data
            gsrc_all = cst.tile([128, IDXCOLS], I16)
            nc.sync.dma_start(out=gsrc_all[:], in_=gsrc[:])
            ld_all = cst.tile([128, LDCOLS], F32)
            nc.sync.dma_start(out=ld_all[:], in_=ldt_d[:])
            bat_all = cst.tile([128, NTILES], F32)
            nc.sync.dma_start(out=bat_all[:], in_=batchf[:])

            xs = slab.tile([128, NPAD], F32)
            nc.sync.dma_start(out=xs[:], in_=xT_own[:])
            xT = [xs[:, t * 128:(t + 1) * 128] for t in range(NTILES)]
            mslab = slab.tile([128, NPAD], F32)
            aggT = [agp.tile([128, 128], F32, tag="aT", name=f"aT{t}")
                    for t in range(NTILES)]

            msg_own = dram.tile([NPAD, D], F32)
            msg_full = dram.tile([TROWS, D], F32, addr_space="Shared")
            ar_in = dram.tile([G, 2], F32)
            ar_out = dram.tile([G, 2], F32, addr_space="Shared")

            # ---- Phase A: msg_own = relu(x_own @ Wm + bm) ----
            for t in range(NTILES):
                pm = ps.tile([128, D], F32, tag="p128")
                nc.tensor.matmul(out=pm[:], lhsT=xT[t],
                                 rhs=wm_t[:], start=True, stop=True)
                mdst = mslab[:, t * 128:(t + 1) * 128]
                nc.vector.tensor_tensor(out=mdst, in0=pm[:], in1=bmt_t[:],
                                        op=mybir.AluOpType.add)
                nc.vector.tensor_scalar_max(out=mdst, in0=mdst, scalar1=0.0)
            # batched store in two chunks so the AllGather can start sooner
            HN = (NTILES // 2) * 128
            nc.sync.dma_start(
                out=msg_own[:HN].rearrange("(t p) d -> p t d", p=128),
                in_=mslab[:, :HN].rearrange("p (t d) -> p t d", d=D))
            nc.sync.dma_start(
                out=msg_own[HN:].rearrange("(t p) d -> p t d", p=128),
                in_=mslab[:, HN:].rearrange("p (t d) -> p t d", d=D))

            # ---- AllGather message table ----
            nc.gpsimd.collective_compute(
                "AllGather", mybir.AluOpType.bypass,
                replica_groups=[list(range(NCORES))],
                ins=[msg_own[:]], outs=[msg_full[:]])

            # ---- Phase B + interleaved C/D ----
            ppool = pool_ps.tile([G, 2], F32)
            qstate = [0]

            def gather(nb, off, half):
                g = gp.tile([128, nb * D], F32, tag="g",
                            name=f"g{qstate[0]}")
                src_t = msg_full[:HALF] if half == 0 else msg_full[HALF:]
                nc.gpsimd.dma_gather(
                    out_ap=g[:].rearrange("p (b d) -> p b d", d=D),
                    in_ap=src_t, idxs_ap=gsrc_all[:, off:off + nb * 8],
                    num_idxs=nb * 128, num_idxs_reg=nb * 128,
                    elem_size=D, single_packet=False,
                    queue_num=qstate[0] % 4)
                qstate[0] += 1
                return g

            for (w, (b_lo, b_hi), (off_lo, off_hi), ld_off) in sched:
                secs = [(gather(b_lo, off_lo, 0), b_lo),
                        (gather(b_hi, off_hi, 1), b_hi)]
                b_tot = b_lo + b_hi
                pagg = psB.tile([128, 128], F32, tag="pagg")
                b = 0
                for (g, nb) in secs:
                    for j in range(nb):
                        s = sp.tile([128, 128], F32, tag="s")
                        col = ld_off + b
                        nc.vector.tensor_tensor(
                            out=s[:],
                            in0=ld_all[:, col:col + 1].to_broadcast([128, 128]),
                            in1=iot[:], op=mybir.AluOpType.is_equal)
                        nc.tensor.matmul(out=pagg[:], lhsT=g[:, j * D:(j + 1) * D],
                                         rhs=s[:], start=(b == 0),
                                         stop=(b == b_tot - 1))
                        b += 1
                nc.vector.tensor_copy(out=aggT[w][:], in_=pagg[:])

                # ---- interleaved Phase C/D for node tile w ----
                t = w
                ph = ps.tile([128, 128], F32, tag="p128")
                nc.tensor.matmul(out=ph[:], lhsT=wux_t[:], rhs=xT[t],
                                 start=True, stop=False)
                nc.tensor.matmul(out=ph[:], lhsT=wua_t[:], rhs=aggT[t][:],
                                 start=False, stop=True)
                ht = cd.tile([128, 128], F32, tag="ht")
                nc.vector.tensor_scalar(out=ht[:], in0=ph[:],
                                        scalar1=bu_t[:, 0:1], scalar2=0.0,
                                        op0=mybir.AluOpType.add,
                                        op1=mybir.AluOpType.max)
                pl = ps.tile([128, 1], F32, tag="p128")
                nc.tensor.matmul(out=pl[:], lhsT=ht[:], rhs=wo_t[:],
                                 start=True, stop=True)
                r2 = cd.tile([128, 2], F32, tag="r2")
                nc.vector.tensor_copy(out=r2[:, 0:1], in_=pl[:])
                nc.vector.tensor_copy(out=r2[:, 1:2], in_=one_c[:])
                sb = sp.tile([128, G], F32, tag="sb")
                nc.vector.tensor_tensor(
                    out=sb[:], in0=bat_all[:, t:t + 1].to_broadcast([128, G]),
                    in1=iot[:, :G], op=mybir.AluOpType.is_equal)
                nc.tensor.matmul(out=ppool[:], lhsT=sb[:], rhs=r2[:],
                                 start=(t == 0), stop=(t == NTILES - 1))

            # ---- AllReduce pooled [64, 2], finish on every core ----
            psb = cd.tile([G, 2], F32, tag="psb")
            nc.vector.tensor_copy(out=psb[:], in_=ppool[:])
            nc.sync.dma_start(out=ar_in[:], in_=psb[:])
            nc.gpsimd.collective_compute(
                "AllReduce", mybir.AluOpType.add,
                replica_groups=[list(range(NCORES))],
                ins=[ar_in[:]], outs=[ar_out[:]])
            fin = cd.tile([G, 2], F32, tag="fin")
            nc.sync.dma_start(out=fin[:], in_=ar_out[:])
            cnt = cd.tile([G, 1], F32, tag="cnt")
            nc.vector.tensor_scalar_max(out=cnt[:], in0=fin[:, 1:2], scalar1=1.0)
            rcp = cd.tile([G, 1], F32, tag="rcp")
            nc.vector.reciprocal(out=rcp[:], in_=cnt[:])
            z = cd.tile([G, 1], F32, tag="z")
            nc.vector.tensor_tensor(out=z[:], in0=fin[:, 0:1], in1=rcp[:],
                                    op=mybir.AluOpType.mult)
            ot = cd.tile([G, 1], F32, tag="ot")
            nc.scalar.activation(out=ot[:], in_=z[:],
                                 func=mybir.ActivationFunctionType.Sigmoid,
                                 bias=bo_t[:])
            nc.sync.dma_start(out=out[:], in_=ot[:])
    nc.finalize()
    return nc


def _run(inputs, trace=False):
    x = np.asarray(inputs["x"], np.float32)
    Wm = np.asarray(inputs["Wm"], np.float32)
    bm = np.asarray(inputs["bm"], np.float32)
    Wu = np.asarray(inputs["Wu"], np.float32)
    bu = np.asarray(inputs["bu"], np.float32)
    Wo = np.asarray(inputs["Wo"], np.float32)
    bo = np.asarray(inputs["bo"], np.float32)

    gsrc, ld, batchf, x_own, sched, IDXCOLS, LDCOLS = _host_prep(
        x, inputs["edge_index"], inputs["batch"])
    nc = _build(sched, IDXCOLS, LDCOLS)

    iota = np.tile(np.arange(128, dtype=np.float32), (128, 1))
    common = {
        "Wm": Wm, "bm_row": bm.reshape(1, D),
        "bm_tile": np.tile(bm.reshape(1, D), (128, 1)),
        "Wu_x": Wu[:D], "Wu_a": Wu[D:], "bu_col": bu.reshape(D, 1),
        "Wo_col": Wo.reshape(D, 1),
        "bo_col": np.full((G, 1), float(bo.reshape(-1)[0]), np.float32),
        "iota": iota,
        "ones_row": np.ones((1, D), np.float32),
        "ones_col": np.ones((128, 1), np.float32),
    }
    in_maps = [dict(common, xT_own=x_own[c], gsrc=gsrc[c], ld=ld[c],
                    batchf=batchf[c]) for c in range(NCORES)]
    res = run_bass_kernel_spmd(nc, in_maps, list(range(NCORES)), trace=trace)
    return res.results[0]["out"].reshape(G, OUT_DIM), res


def kernel(**inputs) -> np.ndarray:
    out, _ = _run(inputs, trace=False)
    return out
